# revision 12
# baseline (speedup 1.0000x reference)
"""Polyphase 2x upsample (scatter into one of 4 phases per batch) + circular
3x3 binomial blur, distributed over 8 TRN2 NeuronCores (data-parallel over
batch: 2 batches per core).

Math: with phase p per batch, r = p % 2, c = p // 2, the reference scatters
x[i,j] to y1[2i+r, 2j+c] (zeros elsewhere) and then blurs with
outer([1,2,1],[1,2,1])/16 under circular padding. The output decomposes into
4 parity classes (all indices mod 128):
  out[2i+r,   2j+c]   = x[i,j] / 4                    (A sites)
  out[2i+r,   2k+1+c] = (x[i,k] + x[i,k+1]) / 8       (H sites)
  out[2i+1+r, 2j+c]   = (x[i,j] + x[i+1,j]) / 8       (V sites)
  out[2i+1+r, 2k+1+c] = sum of the 4 neighbours / 16  (D sites)
With Sx = x + roll_cols(x), t16 = x/16 and Sv = t16 + roll_rows(t16):
  A = 4*t16 (ACT), H = Sx/8 (ACT), V = 2*Sv (ACT), D = Sv[k]+Sv[k+1] (DVE).
All multiplies are powers of two (exact in fp32). Memory-bound shifted-add.

Schedule (the point of this implementation): per-core traffic is 8 MiB in +
32 MiB out; the 16 SDMA engines at ~27 GB/s each (~435 GB/s fabric) put the
DMA roofline at ~98 us, so everything is organized to keep one HWDGE queue
packed end to end:
 - The SP queue carries the four input-chunk loads first (nothing dynamic in
   front of them), then only static contiguous stores. The tiny offs load
   rides the scalar queue.
 - Both the row shift r and the column shift c are folded into the SBUF
   write layout via a 4-arm nested runtime If on p (per batch, DVE+ACT
   only). Stores are fully static [128,64,128] DMAs, two per chunk.
 - For r=1 the V/D row of input pair i lands at out row (2i+2)%128, so each
   half-tile's local row 0 is written from the other half's last pair (seam
   rows); everything stays within the chunk.
 - One If tree per output half with its store directly after: the store is
   released at that half's If-exit (conservative cross-arm semaphore
   accounting gates stores at block exit, so smaller blocks release early).
 - Sx is computed from raw x so the DVE chain does not wait on t16; Sv is
   split wrap-row-first + two halves so half 0's D sites (and the r=1 seam,
   which reads Sv[63]) start before the second half exists.
 - Engine balance per chunk (normal / ~20% throttled): DVE ~16.0 / 19.2 us
   (Sx, t16 at 2x_2P, Sv, D), ACT ~12.9 / 15.5 us (A, H, V) -- both under
   the 19.4 us/chunk store drain rate, so stores never starve even when the
   compute engines run slow. In-arm op order puts each engine's
   earliest-ready producer first.
 - skip_runtime_bounds_check everywhere: the emitted software assert
   instruction faults this runtime.
"""

import sys

for _p in ("/opt/trn_rl_repo",):
    if _p not in sys.path:
        sys.path.insert(0, _p)

import numpy as np

B, C, N = 16, 256, 64
M = 2 * N
NCORES = 8
NB = B // NCORES  # batches per core
NCHUNK = NB * (C // 128)  # [128ch, N, N] chunks per core

_NC_CACHE = None


def _build_nc():
    import concourse.bacc as bacc
    import concourse.bass as bass
    import concourse.mybir as mybir
    import concourse.tile as tile

    f32 = mybir.dt.float32
    i32 = mybir.dt.int32
    add = mybir.AluOpType.add
    ET = mybir.EngineType

    # Bacc (not plain Bass): its finalize() runs generate_event_semaphores,
    # which splits multi-wait instructions — this walrus build allows at
    # most one attached semaphore wait per instruction.
    nc = bacc.Bacc("TRN2", target_bir_lowering=False, debug=False, num_devices=NCORES)
    inp = nc.dram_tensor("inp", [NB, C, N, N], f32, kind="ExternalInput")
    offs = nc.dram_tensor("offs", [1, NB], i32, kind="ExternalInput")
    out = nc.dram_tensor("out", [NB, C, M, M], f32, kind="ExternalOutput")

    with tile.TileContext(nc) as tc:
        with (
            tc.tile_pool(name="offp", bufs=1) as offp,
            tc.tile_pool(name="xp", bufs=3) as xp,
            tc.tile_pool(name="tp", bufs=1) as tp,
            tc.tile_pool(name="op", bufs=3) as op,
        ):
            # SP queue carries only the input loads followed by the static
            # stores; the tiny offs load goes on the scalar HWDGE queue so
            # nothing delays the first load descriptor.
            offs_t = offp.tile([1, NB], i32)
            nc.scalar.dma_start(offs_t[:, :], offs[:, :])

            chunks = [(b, h) for b in range(NB) for h in range(C // 128)]
            xs = []
            for k, (b, h) in enumerate(chunks):
                x = xp.tile([128, N, N], f32, tag="x", name=f"x_{k}")
                nc.sync.dma_start(x[:, :, :], inp[b, 128 * h : 128 * (h + 1)])
                xs.append(x)

            # Phase value per batch, on the two engines that branch.
            pv = [
                nc.values_load(
                    offs_t[0:1, b : b + 1],
                    engines=[ET.DVE, ET.Activation],
                    min_val=0,
                    max_val=3,
                    skip_runtime_bounds_check=True,
                )
                for b in range(NB)
            ]

            def vd_rows(o, rows, Sv, svr, v_cols, d_cols):
                """D sites on DVE (strided shifted add); V sites on ACT
                (scaled copy, V = 2*Sv incl. the j=63 tail inside v_cols)."""
                nc.vector.tensor_tensor(
                    o[:, rows, d_cols],
                    Sv[:, svr, 0:63],
                    Sv[:, svr, 1:64],
                    add,
                )
                nc.scalar.mul(o[:, rows, v_cols], Sv[:, svr, 0:64], 2.0)

            def vd_tails(o, rows, Sv, svr, t_v, t_d):
                # D[63] = Sv[63] + Sv[0] (col wrap); V[63] folded into v_cols
                nc.vector.tensor_tensor(
                    o[:, rows, t_d : t_d + 1],
                    Sv[:, svr, 63:64],
                    Sv[:, svr, 0:1],
                    add,
                )

            def arm_half(o, q, t16, Sx, Sv, r, c):
                """All SBUF writes for output half q (out rows 64q:64q+64)
                of one (r, c) phase. For r=1 this includes the seam: the V/D
                row of pair 32(1-q)+31 lands at this half's local row 0."""
                if c == 0:
                    a_cols = slice(0, 128, 2)   # A at cols 2j
                    hm_cols = slice(1, 128, 2)  # H at 2k+1, k=0..63 (wrap in Sx[63])
                    hm_k = slice(0, 64)
                    hw_cols = None
                    v_cols = slice(0, 128, 2)   # V at 2j, j=0..63
                    d_cols = slice(1, 126, 2)   # D at 2k+1, k=0..62
                    t_v, t_d = 126, 127
                else:
                    a_cols = slice(1, 128, 2)   # A at cols 2j+1
                    hm_cols = slice(2, 128, 2)  # H at 2k+2, k=0..62
                    hm_k = slice(0, 63)
                    hw_cols = slice(0, 1)       # H wrap col (k=63)
                    v_cols = slice(1, 128, 2)   # V at 2j+1, j=0..63
                    d_cols = slice(2, 128, 2)   # D at 2k+2, k=0..62
                    t_v, t_d = 127, 0
                ah_rows = slice(r, 64, 2)       # A/H at local rows 2i'+r
                rs = slice(32 * q, 32 * q + 32)
                # H first (needs only Sx), then A (needs t16): keeps ACT off
                # the store's critical path; V (needs Sv) comes last.
                nc.scalar.mul(o[:, ah_rows, hm_cols], Sx[:, rs, hm_k], 0.125)
                if hw_cols is not None:
                    nc.scalar.mul(o[:, ah_rows, hw_cols], Sx[:, rs, 63:64], 0.125)
                nc.scalar.mul(o[:, ah_rows, a_cols], t16[:, rs, :], 4.0)
                if r == 0:
                    # pair i' -> local odd row 2i'+1
                    vr = slice(1, 64, 2)
                    vd_rows(o, vr, Sv, rs, v_cols, d_cols)
                    vd_tails(o, vr, Sv, rs, t_v, t_d)
                else:
                    # pair i -> out row (2i+2)%128: 31 main rows from this
                    # half's pairs, local row 0 from the other half's last.
                    vr = slice(2, 64, 2)
                    mrs = slice(32 * q, 32 * q + 31)
                    srs = slice(32 * (1 - q) + 31, 32 * (1 - q) + 32)
                    vd_rows(o, vr, Sv, mrs, v_cols, d_cols)
                    vd_tails(o, vr, Sv, mrs, t_v, t_d)
                    sr = slice(0, 1)
                    vd_rows(o, sr, Sv, srs, v_cols, d_cols)
                    vd_tails(o, sr, Sv, srs, t_v, t_d)

            for k, (b, h) in enumerate(chunks):
                x = xs[k]
                chs = slice(128 * h, 128 * (h + 1))
                p = pv[b]

                # Sx[i,k] = x[i,k] + x[i,k+1 mod 64]  (DVE, from raw x so it
                # does not wait on ACT's t16)
                Sx = tp.tile([128, N, N], f32, tag="Sx", name=f"Sx_{k}")
                nc.vector.tensor_tensor(
                    Sx[:, :, 0:63], x[:, :, 0:63], x[:, :, 1:64], add
                )
                nc.vector.tensor_tensor(
                    Sx[:, :, 63:64], x[:, :, 63:64], x[:, :, 0:1], add
                )
                # t16 = x / 16 (DVE: dense fp32 tensor_scalar gets 2x_2P
                # mode there, ~2.2us, vs 3.7us at 1x on ACT; this balances
                # ACT vs DVE per-chunk time under the store drain rate)
                t16 = tp.tile([128, N, N], f32, tag="t16", name=f"t16_{k}")
                nc.vector.tensor_scalar_mul(t16[:, :, :], x[:, :, :], 0.0625)
                # Sv[i,k] = (x[i,k] + x[i+1 mod 64,k]) / 16. Wrap row and
                # first half come first so half-0's V/D (incl. the r=1 seam
                # from Sv[63]) can start before the second half exists.
                Sv = tp.tile([128, N, N], f32, tag="Sv", name=f"Sv_{k}")
                nc.vector.tensor_tensor(
                    Sv[:, 63:64, :], t16[:, 63:64, :], t16[:, 0:1, :], add
                )
                nc.vector.tensor_tensor(
                    Sv[:, 0:32, :], t16[:, 0:32, :], t16[:, 1:33, :], add
                )

                out3 = out[b, chs]  # [128 ch, 128, 128] DRAM view
                # p = r + 2c: 0=(0,0) 1=(1,0) 2=(0,1) 3=(1,1). One If tree
                # per output half with its store right after, so the store
                # is released at that half's If exit, not the chunk's.
                o2 = [
                    op.tile([128, 64, M], f32, tag="o", name=f"o_{k}_{q}")
                    for q in range(2)
                ]
                for q in range(2):
                    if q == 1:
                        nc.vector.tensor_tensor(
                            Sv[:, 32:63, :], t16[:, 32:63, :], t16[:, 33:64, :], add
                        )
                    with tc.If(p < 2) as c_lo:
                        with tc.If(p < 1) as c_p0:
                            arm_half(o2[q], q, t16, Sx, Sv, 0, 0)
                        with c_p0.Else():
                            arm_half(o2[q], q, t16, Sx, Sv, 1, 0)
                    with c_lo.Else():
                        with tc.If(p < 3) as c_p2:
                            arm_half(o2[q], q, t16, Sx, Sv, 0, 1)
                        with c_p2.Else():
                            arm_half(o2[q], q, t16, Sx, Sv, 1, 1)
                    nc.sync.dma_start(
                        out3[:, 64 * q : 64 * (q + 1), :], o2[q][:, :, :]
                    )
    return nc


def _get_nc():
    global _NC_CACHE
    if _NC_CACHE is None:
        _NC_CACHE = _build_nc()
    return _NC_CACHE


def _to_np(a):
    if isinstance(a, np.ndarray):
        return a
    try:
        return np.asarray(a)
    except Exception:
        import jax

        return np.asarray(jax.device_put(a, jax.devices("cpu")[0]))


def kernel(inp, polyphase_indices, _trace=False):
    from concourse.bass_utils import run_bass_kernel_spmd

    inp = np.ascontiguousarray(_to_np(inp), dtype=np.float32)
    idx = _to_np(polyphase_indices).astype(np.int32).reshape(B)
    assert inp.shape == (B, C, N, N)

    in_maps = []
    for k in range(NCORES):
        in_maps.append(
            {
                "inp": np.ascontiguousarray(inp[NB * k : NB * (k + 1)]),
                "offs": np.ascontiguousarray(
                    idx[NB * k : NB * (k + 1)].reshape(1, NB)
                ),
            }
        )

    nc = _get_nc()
    if not nc.is_finalized():
        nc.finalize()
    res = run_bass_kernel_spmd(
        nc, in_maps, core_ids=list(range(NCORES)), trace=_trace
    )
    out = np.concatenate([res.results[k]["out"] for k in range(NCORES)], axis=0)
    if _trace:
        kernel.last_results = res
    return out
